# revision 1
# baseline (speedup 1.0000x reference)
"""MoE++ layer (nn_MoEPlusPlusLayer) on 8 Trainium2 NeuronCores.

Strategy (expert-parallel, per the sharding hint):
  - Host computes the fp32 routing math once to DISPATCH tokens by expert id
    (the sharding step): each of the 8 cores owns one expert's FFN weights and
    receives that expert's assigned tokens as a capacity-padded batch.
  - Gates/biases are folded exactly into the token batch: xg = [g * x^T; g; 0]
    and W1a = [W1; b1; 0], W2a = [W2; b2; 0], using relu(g*z) = g*relu(z) for
    g >= 0. The device FFN is then pure matmul -> relu -> matmul (float32r on
    the PE array at full rate, ~1e-4 relative error).
  - Each core also computes the routing OUTPUTS (router_logits, conf,
    selected_weights, selected_indices) for its 1/8 slice of tokens in fp32
    (data-parallel over the batch, small nets replicated).
  - Host unshards: concatenates routing outputs and combines the two gated
    expert contributions per token with two gathers + add.
"""

import math

import numpy as np

import concourse.bass as bass
import concourse.mybir as mybir
import concourse.tile as tile
from concourse import bacc
from concourse.bass_utils import run_bass_kernel_spmd

# Model dims (fixed by the problem)
B, S, H = 4, 2048, 1024
E, I = 8, 4096
TOP_K = 2
MIN_K, MAX_K = 1, 4
TEMPERATURE = 1.0

N_CORES = 8
N_TOK = B * S                  # 8192
TPC = N_TOK // N_CORES         # tokens per core for the routing math
HC = H // 2                    # confidence hidden dim (512)
KT_H = H // 128                # 8 k-tiles over H
KT_HA = KT_H + 1               # 9 k-tiles over augmented H (bias row)
KT_I = I // 128                # 32 k-tiles over I
KT_IA = KT_I + 1               # 33 k-tiles over augmented I (bias row)
KT_C = HC // 128               # 4 k-tiles over confidence hidden
HA = KT_HA * 128               # 1152 augmented input rows
IA = KT_IA * 128               # 4224 augmented intermediate rows
CHUNK = 512                    # token chunk (= matmul moving dim = psum bank)

F32 = mybir.dt.float32
F32R = mybir.dt.float32r
U32 = mybir.dt.uint32
I32 = mybir.dt.int32
AF = mybir.ActivationFunctionType
AX = mybir.AxisListType
ALU = mybir.AluOpType

_prog_cache: dict = {}


def _build_program(cap: int):
    """One SPMD program, identical on all 8 cores (core e owns expert e)."""
    nc = bacc.Bacc("TRN2", target_bir_lowering=False, debug=False)

    # ---- FFN inputs (per-core = per-expert) ----
    xg = nc.dram_tensor("xg", [HA, cap], F32R, kind="ExternalInput").ap()
    w1a = nc.dram_tensor("w1a", [HA, I], F32R, kind="ExternalInput").ap()
    w2a = nc.dram_tensor("w2a", [IA, H], F32R, kind="ExternalInput").ap()
    # ---- routing inputs (x slice per-core; small nets replicated) ----
    xr = nc.dram_tensor("xr", [H, TPC], F32, kind="ExternalInput").ap()
    wc1 = nc.dram_tensor("wc1", [H, HC], F32, kind="ExternalInput").ap()
    bc1v = nc.dram_tensor("bc1v", [HC], F32, kind="ExternalInput").ap()
    wc2 = nc.dram_tensor("wc2", [HC, 1], F32, kind="ExternalInput").ap()
    bc2b = nc.dram_tensor("bc2b", [128, 1], F32, kind="ExternalInput").ap()
    wr = nc.dram_tensor("wr", [H, E], F32, kind="ExternalInput").ap()
    brb = nc.dram_tensor("brb", [128, E], F32, kind="ExternalInput").ap()

    # ---- outputs ----
    y_out = nc.dram_tensor("y", [H, cap], F32, kind="ExternalOutput").ap()
    rl_out = nc.dram_tensor("rl", [TPC, E], F32, kind="ExternalOutput").ap()
    conf_out = nc.dram_tensor("conf", [TPC, 1], F32, kind="ExternalOutput").ap()
    sw_out = nc.dram_tensor("sw", [TPC, MAX_K], F32, kind="ExternalOutput").ap()
    si_out = nc.dram_tensor("si", [TPC, MAX_K], I32, kind="ExternalOutput").ap()

    n_chunks = cap // CHUNK
    n_tt = TPC // 128  # token tiles for routing

    with tile.TileContext(nc) as tc:
        # ================= routing block (fp32) =================
        with (
            tc.tile_pool(name="rconst", bufs=1) as cp,
            tc.tile_pool(name="rbig", bufs=1) as bp,
            tc.tile_pool(name="rwork", bufs=2) as rp,
            tc.tile_pool(name="rpsum", bufs=2, space="PSUM") as pr,
        ):
            xr_sb = cp.tile([128, KT_H, TPC], F32, tag="xr")
            wc1_sb = cp.tile([128, KT_H, HC], F32, tag="wc1")
            wr_sb = cp.tile([128, KT_H, E], F32, tag="wr")
            wc2_sb = cp.tile([128, KT_C, 1], F32, tag="wc2")
            bc1_sb = cp.tile([128, KT_C], F32, tag="bc1")
            bc2_sb = cp.tile([128, 1], F32, tag="bc2")
            brb_sb = cp.tile([128, E], F32, tag="brb")
            for k in range(KT_H):
                nc.sync.dma_start(xr_sb[:, k], xr[k * 128:(k + 1) * 128, :])
                nc.sync.dma_start(wc1_sb[:, k], wc1[k * 128:(k + 1) * 128, :])
                nc.sync.dma_start(wr_sb[:, k], wr[k * 128:(k + 1) * 128, :])
            for k in range(KT_C):
                nc.sync.dma_start(wc2_sb[:, k], wc2[k * 128:(k + 1) * 128, :])
            nc.sync.dma_start(bc1_sb[:], bc1v.rearrange("(m p) -> p m", p=128))
            nc.sync.dma_start(bc2_sb[:], bc2b[:, :])
            nc.sync.dma_start(brb_sb[:], brb[:, :])

            # conf hidden: c1 = relu(Wc1.T @ x + bc1)   [HC, TPC] feature-major
            c1_sb = bp.tile([128, KT_C, TPC], F32, tag="c1")
            for m in range(KT_C):
                for s in range(TPC // 512):
                    ps = pr.tile([128, 512], F32, tag="ps_c1")
                    for k in range(KT_H):
                        nc.tensor.matmul(
                            ps[:], wc1_sb[:, k, m * 128:(m + 1) * 128],
                            xr_sb[:, k, s * 512:(s + 1) * 512],
                            start=(k == 0), stop=(k == KT_H - 1))
                    nc.scalar.activation(
                        c1_sb[:, m, s * 512:(s + 1) * 512], ps[:], AF.Relu,
                        bias=bc1_sb[:, m:m + 1])

            for t in range(n_tt):
                tsl = slice(t * 128, (t + 1) * 128)
                # router logits (token-major): logits = x_t @ Wr + br
                ps_l = pr.tile([128, E], F32, tag="ps_l")
                for k in range(KT_H):
                    nc.tensor.matmul(ps_l[:], xr_sb[:, k, tsl], wr_sb[:, k],
                                     start=(k == 0), stop=(k == KT_H - 1))
                logit = rp.tile([128, E], F32, tag="logit")
                nc.vector.tensor_tensor(logit[:], ps_l[:], brb_sb[:], ALU.add)
                nc.sync.dma_start(rl_out[tsl, :], logit[:])
                # softmax over E
                mx = rp.tile([128, 1], F32, tag="mx")
                nc.vector.reduce_max(mx[:], logit[:], axis=AX.X)
                nmx = rp.tile([128, 1], F32, tag="nmx")
                nc.vector.tensor_scalar_mul(nmx[:], mx[:], -1.0)
                ex = rp.tile([128, E], F32, tag="ex")
                nc.scalar.activation(ex[:], logit[:], AF.Exp, bias=nmx[:])
                sm = rp.tile([128, 1], F32, tag="sm")
                nc.vector.reduce_sum(sm[:], ex[:], axis=AX.X)
                rs = rp.tile([128, 1], F32, tag="rs")
                nc.vector.reciprocal(rs[:], sm[:])
                probs = rp.tile([128, E], F32, tag="probs")
                nc.vector.tensor_scalar_mul(probs[:], ex[:], rs[:])
                # full sort of the 8 probs (desc) + indices
                sv8 = rp.tile([128, 8], F32, tag="sv8")
                nc.vector.max(sv8[:], probs[:])
                si8 = rp.tile([128, 8], U32, tag="si8")
                nc.vector.max_index(si8[:], sv8[:], probs[:])
                # confidence (token-major): conf = sigmoid(c1_t @ Wc2 + bc2)
                ps_c = pr.tile([128, 1], F32, tag="ps_c")
                for k in range(KT_C):
                    nc.tensor.matmul(ps_c[:], c1_sb[:, k, tsl], wc2_sb[:, k],
                                     start=(k == 0), stop=(k == KT_C - 1))
                conf_t = rp.tile([128, 1], F32, tag="conf_t")
                nc.scalar.activation(conf_t[:], ps_c[:], AF.Sigmoid,
                                     bias=bc2_sb[:])
                nc.sync.dma_start(conf_out[tsl, :], conf_t[:])
                # dyn_k validity: slot k valid iff round(4-3c) > k iff 4-3c >= k+0.5
                v = rp.tile([128, 1], F32, tag="v")
                nc.scalar.activation(v[:], conf_t[:], AF.Copy, bias=4.0,
                                     scale=-3.0)
                sw_t = rp.tile([128, MAX_K], F32, tag="sw_t")
                nc.vector.tensor_copy(sw_t[:], sv8[:, :MAX_K])
                si_f = rp.tile([128, MAX_K], F32, tag="si_f")
                nc.vector.tensor_copy(si_f[:], si8[:, :MAX_K])
                for k in range(1, MAX_K):
                    mk = rp.tile([128, 1], F32, tag="mk")
                    nc.vector.tensor_scalar(mk[:], v[:], float(k) + 0.5,
                                            scalar2=None, op0=ALU.is_ge)
                    nc.vector.tensor_scalar_mul(sw_t[:, k:k + 1],
                                                sw_t[:, k:k + 1], mk[:])
                    nc.vector.tensor_scalar_mul(si_f[:, k:k + 1],
                                                si_f[:, k:k + 1], mk[:])
                si_i = rp.tile([128, MAX_K], I32, tag="si_i")
                nc.vector.tensor_copy(si_i[:], si_f[:])
                nc.sync.dma_start(sw_out[tsl, :], sw_t[:])
                nc.sync.dma_start(si_out[tsl, :], si_i[:])

        # ================= expert FFN block (float32r) =================
        with (
            tc.tile_pool(name="fx", bufs=2) as fx,
            tc.tile_pool(name="fh", bufs=1) as fh,
            tc.tile_pool(name="fw1", bufs=3) as fw1,
            tc.tile_pool(name="fw2", bufs=2) as fw2,
            tc.tile_pool(name="fy", bufs=3) as fy,
            tc.tile_pool(name="fpsum", bufs=4, space="PSUM") as fp,
        ):
            for c in range(n_chunks):
                csl = slice(c * CHUNK, (c + 1) * CHUNK)
                xg_sb = fx.tile([128, KT_HA, CHUNK], F32R, tag="xg")
                for k in range(KT_HA):
                    nc.sync.dma_start(xg_sb[:, k], xg[k * 128:(k + 1) * 128, csl])
                h_sb = fh.tile([128, KT_I, CHUNK], F32R, tag="h")
                for i in range(KT_I):
                    w1t = fw1.tile([128, KT_HA, 128], F32R, tag="w1t")
                    nc.sync.dma_start(
                        w1t[:],
                        w1a[:, i * 128:(i + 1) * 128].rearrange(
                            "(k p) m -> p k m", p=128))
                    ps = fp.tile([128, CHUNK], F32, tag="psA")
                    for k in range(KT_HA):
                        nc.tensor.matmul(ps[:], w1t[:, k], xg_sb[:, k],
                                         start=(k == 0), stop=(k == KT_HA - 1))
                    nc.scalar.activation(h_sb[:, i], ps[:], AF.Relu)
                for hd in range(H // 128):
                    w2t = fw2.tile([128, KT_IA, 128], F32R, tag="w2t")
                    nc.sync.dma_start(
                        w2t[:, :KT_I],
                        w2a[:I, hd * 128:(hd + 1) * 128].rearrange(
                            "(k p) m -> p k m", p=128))
                    nc.sync.dma_start(w2t[:, KT_I],
                                      w2a[I:IA, hd * 128:(hd + 1) * 128])
                    ps2 = fp.tile([128, CHUNK], F32, tag="psB")
                    for k2 in range(KT_I):
                        nc.tensor.matmul(ps2[:], w2t[:, k2], h_sb[:, k2],
                                         start=(k2 == 0), stop=False)
                    # bias row: rhs = [g; 0...] block of xg, lhsT = [b2; 0...]
                    nc.tensor.matmul(ps2[:], w2t[:, KT_I], xg_sb[:, KT_H],
                                     start=False, stop=True)
                    y_sb = fy.tile([128, CHUNK], F32, tag="y")
                    nc.vector.tensor_copy(y_sb[:], ps2[:])
                    nc.sync.dma_start(y_out[hd * 128:(hd + 1) * 128, csl],
                                      y_sb[:])
    nc.compile()
    return nc


def _host_routing(x, Wr, br, Wc1, bc1, Wc2, bc2):
    """fp32 routing on host — used only to DISPATCH tokens to experts."""
    logits = (x @ Wr + br) / TEMPERATURE
    m = logits.max(axis=-1, keepdims=True)
    e = np.exp(logits - m)
    probs = e / e.sum(axis=-1, keepdims=True)
    order = np.argsort(-probs, axis=-1, kind="stable")
    top2 = order[:, :TOP_K]
    bw = np.take_along_axis(probs, top2, axis=-1)
    bwn = bw / bw.sum(axis=-1, keepdims=True)
    return top2.astype(np.int64), bwn.astype(np.float32)


def kernel(hidden_states, Wr, br, Wc1, bc1, Wc2, bc2, W1, b1, W2, b2):
    hidden_states = np.asarray(hidden_states, dtype=np.float32)
    Wr = np.asarray(Wr, np.float32); br = np.asarray(br, np.float32)
    Wc1 = np.asarray(Wc1, np.float32); bc1 = np.asarray(bc1, np.float32)
    Wc2 = np.asarray(Wc2, np.float32); bc2 = np.asarray(bc2, np.float32)
    W1 = np.asarray(W1, np.float32); b1 = np.asarray(b1, np.float32)
    W2 = np.asarray(W2, np.float32); b2 = np.asarray(b2, np.float32)

    x = hidden_states.reshape(-1, H)                       # [N_TOK, H]
    top2, bwn = _host_routing(x, Wr, br, Wc1, bc1, Wc2, bc2)

    # --- dispatch: token/gate lists per expert ---
    toks_e, gates_e = [], []
    for e in range(E):
        m = top2 == e                                      # [N, 2]
        sel = np.nonzero(m.any(axis=1))[0]
        slot = np.argmax(m[sel], axis=1)
        toks_e.append(sel)
        gates_e.append(bwn[sel, slot])
    max_cnt = max(len(t) for t in toks_e)
    cap = max(CHUNK, int(math.ceil(max_cnt / CHUNK)) * CHUNK)

    if cap not in _prog_cache:
        _prog_cache[cap] = _build_program(cap)
    nc = _prog_cache[cap]

    # --- per-core input maps ---
    xT = np.ascontiguousarray(x.T)                         # [H, N_TOK]
    bc2b = np.broadcast_to(bc2.reshape(1, 1), (128, 1)).copy()
    brb = np.broadcast_to(br.reshape(1, E), (128, E)).copy()
    in_maps = []
    for e in range(E):
        toks, g = toks_e[e], gates_e[e]
        cnt = len(toks)
        xg = np.zeros((HA, cap), np.float32)
        xg[:H, :cnt] = xT[:, toks] * g[None, :]
        xg[H, :cnt] = g
        w1a = np.zeros((HA, I), np.float32)
        w1a[:H] = W1[e]
        w1a[H] = b1[e]
        w2a = np.zeros((IA, H), np.float32)
        w2a[:I] = W2[e]
        w2a[I] = b2[e]
        in_maps.append({
            "xg": xg, "w1a": w1a, "w2a": w2a,
            "xr": np.ascontiguousarray(xT[:, e * TPC:(e + 1) * TPC]),
            "wc1": Wc1, "bc1v": bc1, "wc2": Wc2.reshape(HC, 1),
            "bc2b": bc2b, "wr": Wr, "brb": brb,
        })

    results = run_bass_kernel_spmd(nc, in_maps, list(range(N_CORES))).results

    # --- unshard ---
    rl = np.concatenate([r["rl"] for r in results], axis=0)          # [N, E]
    conf = np.concatenate([r["conf"][:, 0] for r in results], axis=0)
    sw = np.concatenate([r["sw"] for r in results], axis=0)
    si = np.concatenate([r["si"] for r in results], axis=0)

    y_cat = np.concatenate([r["y"] for r in results], axis=1)        # [H, E*cap]
    flat = np.zeros((N_TOK, TOP_K), np.int64)
    for e in range(E):
        toks = toks_e[e]
        slot = np.argmax(top2[toks] == e, axis=1)
        flat[toks, slot] = e * cap + np.arange(len(toks))
    out = (y_cat[:, flat[:, 0]] + y_cat[:, flat[:, 1]]).T            # [N, H]

    return (
        out.reshape(B, S, H),
        sw.reshape(B, S, MAX_K),
        si.reshape(B, S, MAX_K).astype(np.int32),
        conf,
        rl,
    )


# revision 4
# speedup vs baseline: 351.1823x; 351.1823x over previous
"""MoE++ layer (nn_MoEPlusPlusLayer) on 8 Trainium2 NeuronCores.

Strategy (expert-parallel, per the sharding hint):
  - Host computes the fp32 routing math once to DISPATCH tokens by expert id
    (the sharding step): each of the 8 cores owns one expert's FFN weights and
    receives that expert's assigned tokens as a capacity-padded batch.
  - Gates/biases are folded exactly into the token batch: xg = [g * x^T; g; 0]
    and W1a = [W1; b1; 0], W2a = [W2; b2; 0], using relu(g*z) = g*relu(z) for
    g >= 0. The device FFN is then pure matmul -> relu -> matmul (float32r on
    the PE array at full rate, ~1e-4 relative error).
  - Each core also computes the routing OUTPUTS (router_logits, conf,
    selected_weights, selected_indices) for its 1/8 slice of tokens in fp32
    (data-parallel over the batch, small nets replicated).
  - Host unshards: concatenates routing outputs and combines the two gated
    expert contributions per token with two gathers + add.
"""

import math

import numpy as np

import concourse.bass as bass
import concourse.mybir as mybir
import concourse.tile as tile
from concourse import bacc
from concourse.bass_utils import run_bass_kernel_spmd

# Model dims (fixed by the problem)
B, S, H = 4, 2048, 1024
E, I = 8, 4096
TOP_K = 2
MIN_K, MAX_K = 1, 4
TEMPERATURE = 1.0

N_CORES = 8
N_TOK = B * S                  # 8192
TPC = N_TOK // N_CORES         # tokens per core for the routing math
HC = H // 2                    # confidence hidden dim (512)
KT_H = H // 128                # 8 k-tiles over H
KT_HA = KT_H + 1               # 9 k-tiles over augmented H (bias row)
KT_I = I // 128                # 32 k-tiles over I
KT_IA = KT_I + 1               # 33 k-tiles over augmented I (bias row)
KT_C = HC // 128               # 4 k-tiles over confidence hidden
HA = KT_HA * 128               # 1152 augmented input rows
IA = KT_IA * 128               # 4224 augmented intermediate rows
CHUNK = 512                    # token chunk (= matmul moving dim = psum bank)

F32 = mybir.dt.float32
F32R = mybir.dt.float32r
U32 = mybir.dt.uint32
I32 = mybir.dt.int32
AF = mybir.ActivationFunctionType
AX = mybir.AxisListType
ALU = mybir.AluOpType

_prog_cache: dict = {}


def _build_program(cap: int):
    """One SPMD program, identical on all 8 cores (core e owns expert e)."""
    nc = bacc.Bacc("TRN2", target_bir_lowering=False, debug=False)

    # ---- FFN inputs (per-core = per-expert) ----
    xg = nc.dram_tensor("xg", [HA, cap], F32R, kind="ExternalInput").ap()
    w1a = nc.dram_tensor("w1a", [HA, I], F32R, kind="ExternalInput").ap()
    w2a = nc.dram_tensor("w2a", [IA, H], F32R, kind="ExternalInput").ap()
    # ---- routing inputs (x slice per-core; small nets replicated) ----
    xr = nc.dram_tensor("xr", [H, TPC], F32, kind="ExternalInput").ap()
    wc1 = nc.dram_tensor("wc1", [H, HC], F32, kind="ExternalInput").ap()
    bc1v = nc.dram_tensor("bc1v", [HC], F32, kind="ExternalInput").ap()
    wc2 = nc.dram_tensor("wc2", [HC, 1], F32, kind="ExternalInput").ap()
    bc2b = nc.dram_tensor("bc2b", [128, 1], F32, kind="ExternalInput").ap()
    wr = nc.dram_tensor("wr", [H, E], F32, kind="ExternalInput").ap()
    brb = nc.dram_tensor("brb", [128, E], F32, kind="ExternalInput").ap()

    # ---- outputs ----
    y_out = nc.dram_tensor("y", [H, cap], F32, kind="ExternalOutput").ap()
    rl_out = nc.dram_tensor("rl", [TPC, E], F32, kind="ExternalOutput").ap()
    conf_out = nc.dram_tensor("conf", [TPC, 1], F32, kind="ExternalOutput").ap()
    sw_out = nc.dram_tensor("sw", [TPC, MAX_K], F32, kind="ExternalOutput").ap()
    si_out = nc.dram_tensor("si", [TPC, MAX_K], I32, kind="ExternalOutput").ap()

    n_chunks = cap // CHUNK
    n_tt = TPC // 128  # token tiles for routing

    with tile.TileContext(nc) as tc:
        # ================= routing block (fp32) =================
        with (
            tc.tile_pool(name="rconst", bufs=1) as cp,
            tc.tile_pool(name="rbig", bufs=1) as bp,
            tc.tile_pool(name="rwork", bufs=2) as rp,
            tc.tile_pool(name="rpsum", bufs=2, space="PSUM") as pr,
        ):
            xr_sb = cp.tile([128, KT_H, TPC], F32, tag="xr")
            wc1_sb = cp.tile([128, KT_H, HC], F32, tag="wc1")
            wr_sb = cp.tile([128, KT_H, E], F32, tag="wr")
            wc2_sb = cp.tile([128, KT_C, 1], F32, tag="wc2")
            bc1_sb = cp.tile([128, KT_C], F32, tag="bc1")
            bc2_sb = cp.tile([128, 1], F32, tag="bc2")
            brb_sb = cp.tile([128, E], F32, tag="brb")
            for k in range(KT_H):
                nc.sync.dma_start(xr_sb[:, k], xr[k * 128:(k + 1) * 128, :])
                nc.sync.dma_start(wc1_sb[:, k], wc1[k * 128:(k + 1) * 128, :])
                nc.sync.dma_start(wr_sb[:, k], wr[k * 128:(k + 1) * 128, :])
            for k in range(KT_C):
                nc.sync.dma_start(wc2_sb[:, k], wc2[k * 128:(k + 1) * 128, :])
            nc.sync.dma_start(bc1_sb[:], bc1v.rearrange("(m p) -> p m", p=128))
            nc.sync.dma_start(bc2_sb[:], bc2b[:, :])
            nc.sync.dma_start(brb_sb[:], brb[:, :])

            # conf hidden: c1 = relu(Wc1.T @ x + bc1)   [HC, TPC] feature-major
            c1_sb = bp.tile([128, KT_C, TPC], F32, tag="c1")
            for m in range(KT_C):
                for s in range(TPC // 512):
                    ps = pr.tile([128, 512], F32, tag="ps_c1")
                    for k in range(KT_H):
                        nc.tensor.matmul(
                            ps[:], wc1_sb[:, k, m * 128:(m + 1) * 128],
                            xr_sb[:, k, s * 512:(s + 1) * 512],
                            start=(k == 0), stop=(k == KT_H - 1))
                    nc.scalar.activation(
                        c1_sb[:, m, s * 512:(s + 1) * 512], ps[:], AF.Relu,
                        bias=bc1_sb[:, m:m + 1])

            for t in range(n_tt):
                tsl = slice(t * 128, (t + 1) * 128)
                # router logits (token-major): logits = x_t @ Wr + br
                ps_l = pr.tile([128, E], F32, tag="ps_l")
                for k in range(KT_H):
                    nc.tensor.matmul(ps_l[:], xr_sb[:, k, tsl], wr_sb[:, k],
                                     start=(k == 0), stop=(k == KT_H - 1))
                logit = rp.tile([128, E], F32, tag="logit")
                nc.vector.tensor_tensor(logit[:], ps_l[:], brb_sb[:], ALU.add)
                nc.sync.dma_start(rl_out[tsl, :], logit[:])
                # softmax over E
                mx = rp.tile([128, 1], F32, tag="mx")
                nc.vector.reduce_max(mx[:], logit[:], axis=AX.X)
                nmx = rp.tile([128, 1], F32, tag="nmx")
                nc.vector.tensor_scalar_mul(nmx[:], mx[:], -1.0)
                ex = rp.tile([128, E], F32, tag="ex")
                nc.scalar.activation(ex[:], logit[:], AF.Exp, bias=nmx[:])
                sm = rp.tile([128, 1], F32, tag="sm")
                nc.vector.reduce_sum(sm[:], ex[:], axis=AX.X)
                rs = rp.tile([128, 1], F32, tag="rs")
                nc.vector.reciprocal(rs[:], sm[:])
                probs = rp.tile([128, E], F32, tag="probs")
                nc.vector.tensor_scalar_mul(probs[:], ex[:], rs[:])
                # full sort of the 8 probs (desc) + indices
                sv8 = rp.tile([128, 8], F32, tag="sv8")
                nc.vector.max(sv8[:], probs[:])
                si8 = rp.tile([128, 8], U32, tag="si8")
                nc.vector.max_index(si8[:], sv8[:], probs[:])
                # confidence (token-major): conf = sigmoid(c1_t @ Wc2 + bc2)
                ps_c = pr.tile([128, 1], F32, tag="ps_c")
                for k in range(KT_C):
                    nc.tensor.matmul(ps_c[:], c1_sb[:, k, tsl], wc2_sb[:, k],
                                     start=(k == 0), stop=(k == KT_C - 1))
                conf_t = rp.tile([128, 1], F32, tag="conf_t")
                nc.scalar.activation(conf_t[:], ps_c[:], AF.Sigmoid,
                                     bias=bc2_sb[:])
                nc.sync.dma_start(conf_out[tsl, :], conf_t[:])
                # dyn_k validity: slot k valid iff round(4-3c) > k iff 4-3c >= k+0.5
                v = rp.tile([128, 1], F32, tag="v")
                nc.scalar.activation(v[:], conf_t[:], AF.Copy, bias=4.0,
                                     scale=-3.0)
                sw_t = rp.tile([128, MAX_K], F32, tag="sw_t")
                nc.vector.tensor_copy(sw_t[:], sv8[:, :MAX_K])
                si_f = rp.tile([128, MAX_K], F32, tag="si_f")
                nc.vector.tensor_copy(si_f[:], si8[:, :MAX_K])
                for k in range(1, MAX_K):
                    mk = rp.tile([128, 1], F32, tag="mk")
                    nc.vector.tensor_scalar(mk[:], v[:], float(k) + 0.5,
                                            scalar2=None, op0=ALU.is_ge)
                    nc.vector.tensor_scalar_mul(sw_t[:, k:k + 1],
                                                sw_t[:, k:k + 1], mk[:])
                    nc.vector.tensor_scalar_mul(si_f[:, k:k + 1],
                                                si_f[:, k:k + 1], mk[:])
                si_i = rp.tile([128, MAX_K], I32, tag="si_i")
                nc.vector.tensor_copy(si_i[:], si_f[:])
                nc.sync.dma_start(sw_out[tsl, :], sw_t[:])
                nc.sync.dma_start(si_out[tsl, :], si_i[:])

        # ================= expert FFN block (float32r) =================
        with (
            tc.tile_pool(name="fx", bufs=2) as fx,
            tc.tile_pool(name="fh", bufs=1) as fh,
            tc.tile_pool(name="fw1", bufs=3) as fw1,
            tc.tile_pool(name="fw2", bufs=2) as fw2,
            tc.tile_pool(name="fy", bufs=3) as fy,
            tc.tile_pool(name="fpsum", bufs=4, space="PSUM") as fp,
        ):
            for c in range(n_chunks):
                csl = slice(c * CHUNK, (c + 1) * CHUNK)
                xg_sb = fx.tile([128, KT_HA, CHUNK], F32R, tag="xg")
                for k in range(KT_HA):
                    nc.sync.dma_start(xg_sb[:, k], xg[k * 128:(k + 1) * 128, csl])
                h_sb = fh.tile([128, KT_I, CHUNK], F32R, tag="h")
                for i in range(KT_I):
                    w1t = fw1.tile([128, KT_HA, 128], F32R, tag="w1t")
                    nc.sync.dma_start(
                        w1t[:],
                        w1a[:, i * 128:(i + 1) * 128].rearrange(
                            "(k p) m -> p k m", p=128))
                    ps = fp.tile([128, CHUNK], F32, tag="psA")
                    for k in range(KT_HA):
                        nc.tensor.matmul(ps[:], w1t[:, k], xg_sb[:, k],
                                         start=(k == 0), stop=(k == KT_HA - 1))
                    nc.scalar.activation(h_sb[:, i], ps[:], AF.Relu)
                for hd in range(H // 128):
                    w2t = fw2.tile([128, KT_IA, 128], F32R, tag="w2t")
                    nc.sync.dma_start(
                        w2t[:, :KT_I],
                        w2a[:I, hd * 128:(hd + 1) * 128].rearrange(
                            "(k p) m -> p k m", p=128))
                    nc.sync.dma_start(w2t[:, KT_I],
                                      w2a[I:IA, hd * 128:(hd + 1) * 128])
                    ps2 = fp.tile([128, CHUNK], F32, tag="psB")
                    for k2 in range(KT_I):
                        nc.tensor.matmul(ps2[:], w2t[:, k2], h_sb[:, k2],
                                         start=(k2 == 0), stop=False)
                    # bias row: rhs = [g; 0...] block of xg, lhsT = [b2; 0...]
                    nc.tensor.matmul(ps2[:], w2t[:, KT_I], xg_sb[:, KT_H],
                                     start=False, stop=True)
                    y_sb = fy.tile([128, CHUNK], F32, tag="y")
                    nc.vector.tensor_copy(y_sb[:], ps2[:])
                    nc.sync.dma_start(y_out[hd * 128:(hd + 1) * 128, csl],
                                      y_sb[:])
    nc.compile()
    return nc


def _host_routing(x, Wr, br, Wc1, bc1, Wc2, bc2):
    """fp32 routing on host — used only to DISPATCH tokens to experts."""
    logits = (x @ Wr + br) / TEMPERATURE
    m = logits.max(axis=-1, keepdims=True)
    e = np.exp(logits - m)
    probs = e / e.sum(axis=-1, keepdims=True)
    order = np.argsort(-probs, axis=-1, kind="stable")
    top2 = order[:, :TOP_K]
    bw = np.take_along_axis(probs, top2, axis=-1)
    bwn = bw / bw.sum(axis=-1, keepdims=True)
    return top2.astype(np.int64), bwn.astype(np.float32)


def prepare(hidden_states, Wr, br, Wc1, bc1, Wc2, bc2, W1, b1, W2, b2):
    """Host sharding: routing + dispatch + per-core input maps.
    Returns (nc, in_maps, meta) where meta is needed by combine()."""
    hidden_states = np.asarray(hidden_states, dtype=np.float32)
    Wr = np.asarray(Wr, np.float32); br = np.asarray(br, np.float32)
    Wc1 = np.asarray(Wc1, np.float32); bc1 = np.asarray(bc1, np.float32)
    Wc2 = np.asarray(Wc2, np.float32); bc2 = np.asarray(bc2, np.float32)
    W1 = np.asarray(W1, np.float32); b1 = np.asarray(b1, np.float32)
    W2 = np.asarray(W2, np.float32); b2 = np.asarray(b2, np.float32)

    x = hidden_states.reshape(-1, H)                       # [N_TOK, H]
    top2, bwn = _host_routing(x, Wr, br, Wc1, bc1, Wc2, bc2)

    # --- dispatch: token/gate lists per expert ---
    toks_e, gates_e = [], []
    for e in range(E):
        m = top2 == e                                      # [N, 2]
        sel = np.nonzero(m.any(axis=1))[0]
        slot = np.argmax(m[sel], axis=1)
        toks_e.append(sel)
        gates_e.append(bwn[sel, slot])
    max_cnt = max(len(t) for t in toks_e)
    cap = max(CHUNK, int(math.ceil(max_cnt / CHUNK)) * CHUNK)

    if cap not in _prog_cache:
        _prog_cache[cap] = _build_program(cap)
    nc = _prog_cache[cap]

    # --- per-core input maps ---
    xT = np.ascontiguousarray(x.T)                         # [H, N_TOK]
    bc2b = np.broadcast_to(bc2.reshape(1, 1), (128, 1)).copy()
    brb = np.broadcast_to(br.reshape(1, E), (128, E)).copy()
    in_maps = []
    for e in range(E):
        toks, g = toks_e[e], gates_e[e]
        cnt = len(toks)
        xg = np.zeros((HA, cap), np.float32)
        xg[:H, :cnt] = xT[:, toks] * g[None, :]
        xg[H, :cnt] = g
        w1a = np.zeros((HA, I), np.float32)
        w1a[:H] = W1[e]
        w1a[H] = b1[e]
        w2a = np.zeros((IA, H), np.float32)
        w2a[:I] = W2[e]
        w2a[I] = b2[e]
        in_maps.append({
            "xg": xg, "w1a": w1a, "w2a": w2a,
            "xr": np.ascontiguousarray(xT[:, e * TPC:(e + 1) * TPC]),
            "wc1": Wc1, "bc1v": bc1, "wc2": Wc2.reshape(HC, 1),
            "bc2b": bc2b, "wr": Wr, "brb": brb,
        })
    return nc, in_maps, (toks_e, top2, cap)


def combine(results, meta):
    """Unshard the per-core results into the full reference-shaped outputs."""
    toks_e, top2, cap = meta
    # --- unshard ---
    rl = np.concatenate([r["rl"] for r in results], axis=0)          # [N, E]
    conf = np.concatenate([r["conf"][:, 0] for r in results], axis=0)
    sw = np.concatenate([r["sw"] for r in results], axis=0)
    si = np.concatenate([r["si"] for r in results], axis=0)

    y_cat = np.concatenate([r["y"] for r in results], axis=1)        # [H, E*cap]
    flat = np.zeros((N_TOK, TOP_K), np.int64)
    for e in range(E):
        toks = toks_e[e]
        slot = np.argmax(top2[toks] == e, axis=1)
        flat[toks, slot] = e * cap + np.arange(len(toks))
    out = (y_cat[:, flat[:, 0]] + y_cat[:, flat[:, 1]]).T            # [N, H]

    return (
        out.reshape(B, S, H),
        sw.reshape(B, S, MAX_K),
        si.reshape(B, S, MAX_K).astype(np.int32),
        conf,
        rl,
    )


def kernel(**inputs):
    nc, in_maps, meta = prepare(**inputs)
    results = run_bass_kernel_spmd(nc, in_maps, list(range(N_CORES))).results
    return combine(results, meta)
